# revision 1
# baseline (speedup 1.0000x reference)
"""Trainium2 Bass kernel for nn_AE_KGCN (AE encoder + KGCN attention + tied decoder).

Sharding: items (25000) and enc_w0 vocab-columns (25274) are co-sharded over 8
cores (3125 items + ~3160 vocab cols each, padded to 3200). One AllReduce of the
[64,512] encoder partial sums is the only collective.

Attention mapping per core (3136 padded items, 98 tiles of 32 items):
  scores  s[(m,n),b] = nbr_rT-chunk.T @ userT           (PE, K=32)
  E       = exp(s/32)                                   (ACT, psum->sbuf bf16)
  denom   rep[(m,n),b] = SEL.T @ E  (SEL = 4x4 block-diag ones)   (PE)
  rcp     = approx_recip(denom)                          (DVE custom op)
  En      = E * rcp          (softmax weights, sum_n = 1) (DVE)
  numer   [(g,b),(m,d)] = En-tile.T @ block-diag(P') , P' = nbr_e@fc2.T/4+iproj
          (PE, 2-col-tiled: item-tile A -> psum rows 0:64, B -> 64:128)
  tanh    (ACT) ; TU = T * user (gpsimd) ; ret = sum_d TU (DVE reduce)
  out     = sigmoid(zdec + ret)  fused in decode chunks.
"""

import sys

for p in ("/opt/trn_rl_repo", "/opt/pypackages"):
    if p not in sys.path:
        sys.path.insert(0, p)

import numpy as np
import ml_dtypes
import concourse.bass as bass
import concourse.mybir as mybir
import concourse.tile as tile
import concourse.bacc as bacc
from concourse.bass_utils import run_bass_kernel_spmd
from concourse.dve_ops import RECIPROCAL_APPROX_FAST, RECIP_APPROX_FAST_CONSTS

F32 = mybir.dt.float32
BF16 = mybir.dt.bfloat16
AX = mybir.AxisListType
ALU = mybir.AluOpType
ACTF = mybir.ActivationFunctionType

B = 64
NV = 25274
NI = 25000
DIM = 32
NN = 4
H1 = 512
H2 = 64
NC = 8
MS = NI // NC            # 3125 items per core
NT = 98                  # 32-item tiles per core
MSP = NT * 32            # 3136 padded items
NST = NT // 2            # 49 supertiles (64 items)
VCP = 3200               # padded vocab cols per core (25*128)
NVCH = VCP // 128        # 25 encoder K-chunks
GB = 14                  # BDP tiles per DMA batch
NB = NT // GB            # 7 batches
SELU_L = 1.0507009873554805
SELU_A = 1.6732632423543772
BN_EPS = 1e-5

_CACHE = {}


def _build_graph():
    nc = bacc.Bacc("TRN2", target_bir_lowering=False, debug=False,
                   enable_asserts=False, num_devices=NC)

    def din(name, shape, dt=BF16):
        return nc.dram_tensor(name, shape, dt, kind="ExternalInput").ap()

    xT = din("xT", [VCP, B])                 # x shard transposed, row 3199 = 1.0
    w0e = din("w0e", [VCP, H1])              # enc_w0 shard .T, row 3199 = enc_b0/8
    w0d = din("w0d", [4, 128, VCP])          # enc_w0 shard, h1-chunked
    db1 = din("db1", [1, VCP])               # dec_b1 shard row
    w1T = din("w1T", [H1, H2])               # enc_w1.T
    b1r = din("b1r", [1, H2])                # enc_b1 row
    w1 = din("w1", [H2, H1])                 # enc_w1
    db0 = din("db0", [1, H1])                # dec_b0 row
    uwT = din("uwT", [H2, DIM])              # u_w.T
    ubr = din("ubr", [1, DIM])               # u_b row
    nrT = din("nrT", [DIM, MSP * NN])        # nbr_rel shard [d, (m,n)]
    bdpd = din("bdpd", [NB, 2, 16, NN, GB, DIM])  # diag-packed P'
    selc = din("selc", [128, 128])           # block-diag 4x4 ones selector
    ones1 = din("ones1", [1, B])             # K=1 bias-row lhsT
    ident = din("ident", [B, B])             # identity for PE transpose
    gbT = din("gbT", [4, 128, 2], F32)       # [gamma, beta] per h1, chunked
    zrow = din("zrow", [1, 1024])            # zeros row for BDP clearing
    out_d = nc.dram_tensor("out", [B, VCP], F32, kind="ExternalOutput").ap()

    from contextlib import ExitStack
    with tile.TileContext(nc) as tc, ExitStack() as ctx:
        sb = ctx.enter_context(tc.tile_pool(name="sb", bufs=2))
        sb3 = ctx.enter_context(tc.tile_pool(name="sb3", bufs=3))
        sb1 = ctx.enter_context(tc.tile_pool(name="sb1", bufs=1))
        ps = ctx.enter_context(tc.tile_pool(name="ps", bufs=3, space="PSUM"))
        ps1 = ctx.enter_context(tc.tile_pool(name="ps1", bufs=1, space="PSUM"))
        psm = ctx.enter_context(tc.tile_pool(name="psm", bufs=2, space="PSUM"))
        dram = ctx.enter_context(tc.tile_pool(name="dram", bufs=1, space="DRAM"))

        # ---- persistent SBUF ----
        xT_sb = sb1.tile([128, NVCH * B], BF16, tag="xT")
        w0d_sb = sb1.tile([128, 4 * VCP], BF16, tag="w0d")
        db1_sb = sb1.tile([1, VCP], BF16, tag="db1")
        w1T_sb = sb1.tile([128, 4 * H2], BF16, tag="w1T")
        w1_sb = sb1.tile([H2, H1], BF16, tag="w1")
        db0_sb = sb1.tile([1, H1], BF16, tag="db0")
        b1r_sb = sb1.tile([1, H2], BF16, tag="b1r")
        uwT_sb = sb1.tile([H2, DIM], BF16, tag="uwT")
        ubr_sb = sb1.tile([1, DIM], BF16, tag="ubr")
        nrT_sb = sb1.tile([DIM, MSP * NN], BF16, tag="nrT")
        selc_sb = sb1.tile([128, 128], BF16, tag="selc")
        ones1_sb = sb1.tile([1, B], BF16, tag="ones1")
        ident_sb = sb1.tile([B, B], BF16, tag="ident")
        gbT_sb = sb1.tile([128, 4 * 2], F32, tag="gbT")
        En_sb = sb1.tile([128, NT * B], BF16, tag="En")
        ret_sb = sb1.tile([128, NST * 32], F32, tag="ret")
        ret2_sb = sb1.tile([B, NST * 32], F32, tag="ret2")
        hT_sb = sb1.tile([128, 4 * B], BF16, tag="hT")
        u2_sb = sb1.tile([128, DIM], BF16, tag="u2")
        usrT_sb = sb1.tile([DIM, B], BF16, tag="usrT")
        zbnT_sb = sb1.tile([128, 4 * B], BF16, tag="zbnT")
        bdp0 = sb1.tile([128, GB * 1024], BF16, tag="bdp0")
        bdp1 = sb1.tile([128, GB * 1024], BF16, tag="bdp1")
        bdps = [bdp0, bdp1]

        # ---- encoder-critical + tiny input DMAs first ----
        nc.sync.dma_start(xT_sb[:].rearrange("p (c b) -> p c b", b=B), xT.rearrange("(c p) b -> c p b", p=128).transpose([1, 0, 2]))
        nc.sync.dma_start(w1T_sb[:].rearrange("p (c h) -> p c h", h=H2), w1T.rearrange("(c p) h -> c p h", p=128).transpose([1, 0, 2]))
        nc.sync.dma_start(w1_sb[:], w1[:])
        nc.sync.dma_start(db0_sb[:], db0[:])
        nc.sync.dma_start(b1r_sb[:], b1r[:])
        nc.sync.dma_start(uwT_sb[:], uwT[:])
        nc.sync.dma_start(ubr_sb[:], ubr[:])
        nc.sync.dma_start(selc_sb[:], selc[:])
        nc.sync.dma_start(ones1_sb[:], ones1[:])
        nc.sync.dma_start(ident_sb[:], ident[:])
        nc.sync.dma_start(gbT_sb[:].rearrange("p (c t) -> p c t", t=2), gbT.transpose([1, 0, 2]))

        def bdp_load(q):
            buf = bdps[q % 2]
            for h in range(2):
                for j in range(16):
                    dst = buf[64 * h + 4 * j:64 * h + 4 * j + 4, :] \
                        .rearrange("n (t x d) -> n t x d", x=32, d=DIM)[:, :, h * 16 + j, :]
                    nc.sync.dma_start(dst, bdpd[q, h, j])

        def bulk_loads():
            nc.sync.dma_start(nrT_sb[:], nrT[:])
            zsrc = zrow.unsqueeze(0).broadcast_to([128, GB, 1024])
            nc.sync.dma_start(bdp0[:].rearrange("p (t x) -> p t x", x=1024), zsrc)
            nc.sync.dma_start(bdp1[:].rearrange("p (t x) -> p t x", x=1024), zsrc)
            bdp_load(0)
            bdp_load(1)
            nc.sync.dma_start(w0d_sb[:].rearrange("p (c v) -> p c v", v=VCP), w0d.transpose([1, 0, 2]))
            nc.sync.dma_start(db1_sb[:], db1[:])

        # ================= encoder =================
        h1ps = ps1.tile([B, H1], F32, tag="misc")
        w0ev = w0e.rearrange("(c p) h -> c p h", p=128)
        for v in range(NVCH):
            w0ec = sb3.tile([128, H1], BF16, tag="w0ec")
            nc.sync.dma_start(w0ec[:], w0ev[v])
            nc.tensor.matmul(
                h1ps[:], xT_sb[:, v * B:(v + 1) * B], w0ec[:],
                start=(v == 0), stop=(v == NVCH - 1))
        h1sb = sb.tile([B, H1], F32, tag="h1sb")
        nc.scalar.copy(h1sb[:], h1ps[:])

        bnc_in = dram.tile([B, H1], F32)
        bnc_out = dram.tile([B, H1], F32)
        nc.sync.dma_start(bnc_in[:], h1sb[:])
        nc.gpsimd.collective_compute(
            "AllReduce", ALU.add, replica_groups=[list(range(NC))],
            ins=[bnc_in.opt()], outs=[bnc_out.opt()])
        bulk_loads()
        h1r = sb.tile([B, H1], F32, tag="h1r")
        nc.sync.dma_start(h1r[:], bnc_out[:])

        # selu helper: dst = SL*relu(x) + min(SA*SL*(exp(x)-1), 0)
        def selu(dst, src, P, W, tagp="sl"):
            e = sb.tile([P, W], F32, tag=tagp + "e")
            t = sb.tile([P, W], F32, tag=tagp + "t")
            f = sb.tile([P, W], F32, tag=tagp + "f")
            nc.scalar.activation(e[:], src, ACTF.Exp)
            nc.vector.tensor_scalar(t[:], src, SELU_L, 0.0, op0=ALU.mult, op1=ALU.max)
            nc.vector.tensor_scalar(f[:], e[:], SELU_A * SELU_L, -SELU_A * SELU_L,
                                    op0=ALU.mult, op1=ALU.add)
            nc.vector.tensor_scalar(f[:], f[:], 0.0, None, op0=ALU.min)
            nc.vector.tensor_tensor(dst, t[:], f[:], op=ALU.add)

        h_sb = sb.tile([B, H1], BF16, tag="h")
        selu(h_sb[:], h1r[:], B, H1)
        # hT via 4 PE transposes
        for i in range(4):
            htp = ps1.tile([128, B], BF16, tag="misc")
            nc.tensor.transpose(htp[:], h_sb[:, 128 * i:128 * (i + 1)], ident_sb[:])
            nc.scalar.copy(hT_sb[:, i * B:(i + 1) * B], htp[:])

        # ================= h2 / user / z =================
        h2ps = ps1.tile([B, H2], F32, tag="misc")
        for k in range(4):
            nc.tensor.matmul(h2ps[:], hT_sb[:, k * B:(k + 1) * B],
                             w1T_sb[:, k * H2:(k + 1) * H2],
                             start=(k == 0), stop=False)
        nc.tensor.matmul(h2ps[:], ones1_sb[:], b1r_sb[:], start=False, stop=True)
        h2s = sb.tile([B, H2], BF16, tag="h2s")
        selu(h2s[:], h2ps[:], B, H2)
        h2sT_ps = ps1.tile([H2, B], BF16, tag="misc")
        nc.tensor.transpose(h2sT_ps[:], h2s[:], ident_sb[:])
        h2sT = sb.tile([H2, B], BF16, tag="h2sTs")
        nc.scalar.copy(h2sT[:], h2sT_ps[:])

        usr_ps = ps1.tile([B, DIM], F32, tag="misc")
        nc.tensor.matmul(usr_ps[:], h2sT[:], uwT_sb[:], start=True, stop=False)
        nc.tensor.matmul(usr_ps[:], ones1_sb[:], ubr_sb[:], start=False, stop=True)
        usr_sb = sb.tile([B, DIM], BF16, tag="usrsb")
        nc.scalar.copy(usr_sb[:], usr_ps[:])
        usrT_ps = ps1.tile([DIM, B], BF16, tag="misc")
        nc.tensor.transpose(usrT_ps[:], usr_sb[:], ident_sb[:])
        nc.scalar.copy(usrT_sb[:], usrT_ps[:])
        nc.sync.dma_start(u2_sb[0:B, :], usr_sb[:])
        nc.sync.dma_start(u2_sb[B:128, :], usr_sb[:])

        zps = ps1.tile([B, H1], F32, tag="misc")
        nc.tensor.matmul(zps[:], h2sT[:], w1_sb[:], start=True, stop=False)
        nc.tensor.matmul(zps[:], ones1_sb[:], db0_sb[:], start=False, stop=True)
        z_sb = sb.tile([B, H1], BF16, tag="zsb")
        selu(z_sb[:], zps[:], B, H1)

        # ============ scores + exp + denom + recip + E-norm (per 8-tile group) ====
        for g in range(13):  # 13 groups of 8 tiles (98 tiles)
            t0, t1 = g * 8, min(g * 8 + 8, NT)
            nt = t1 - t0
            sps = ps.tile([128, 512], F32, tag="sd")
            for t in range(t0, t1):
                nc.tensor.matmul(sps[:, (t - t0) * B:(t - t0 + 1) * B],
                                 nrT_sb[:, t * 128:(t + 1) * 128], usrT_sb[:],
                                 start=True, stop=True)
            Eg = sb3.tile([128, 512], BF16, tag="Eg")
            nc.scalar.activation(Eg[:, :nt * B], sps[:, :nt * B],
                                 ACTF.Exp, scale=1.0 / DIM)
            dps = ps.tile([128, 512], F32, tag="sd")
            for t in range(t0, t1):
                nc.tensor.matmul(dps[:, (t - t0) * B:(t - t0 + 1) * B],
                                 selc_sb[:], Eg[:, (t - t0) * B:(t - t0 + 1) * B],
                                 start=True, stop=True)
            rcpg = sb3.tile([128, 512], BF16, tag="rcpg")
            nc.vector._custom_dve(
                RECIPROCAL_APPROX_FAST,
                out=rcpg[:, :nt * B], in0=dps[:, :nt * B],
                s0=RECIP_APPROX_FAST_CONSTS["s0"], s1=RECIP_APPROX_FAST_CONSTS["s1"],
                imm2=RECIP_APPROX_FAST_CONSTS["imm2"])
            nc.vector.tensor_tensor(En_sb[:, t0 * B:t1 * B], Eg[:, :nt * B],
                                    rcpg[:, :nt * B], op=ALU.mult)

        # ================= numer MMs + tanh + TU + reduce =================
        for st in range(NST):
            ta, tb = 2 * st, 2 * st + 1
            q, ia = ta // GB, ta % GB
            ib = tb % GB
            nps = psm.tile([128, 1024], F32, tag="nps")
            buf = bdps[q % 2]
            for half in range(2):
                nc.tensor.matmul(nps[0:B, half * 512:(half + 1) * 512],
                                 En_sb[:, ta * B:(ta + 1) * B],
                                 buf[:, ia * 1024 + half * 512: ia * 1024 + (half + 1) * 512],
                                 start=True, stop=True, tile_position=(0, 0))
                nc.tensor.matmul(nps[B:128, half * 512:(half + 1) * 512],
                                 En_sb[:, tb * B:(tb + 1) * B],
                                 buf[:, ib * 1024 + half * 512: ib * 1024 + (half + 1) * 512],
                                 start=True, stop=True, tile_position=(0, B))
            if ia == GB - 2 and q + 2 < NB:
                bdp_load(q + 2)
            T_sb = sb3.tile([128, 1024], BF16, tag="T")
            nc.scalar.activation(T_sb[:], nps[:], ACTF.Tanh)
            TU = sb3.tile([128, 1024], BF16, tag="TU")
            tt_eng = nc.vector if (st % 2 == 0) else nc.gpsimd
            tt_eng.tensor_tensor(
                TU[:].rearrange("p (m d) -> p m d", d=DIM),
                T_sb[:].rearrange("p (m d) -> p m d", d=DIM),
                u2_sb[:].unsqueeze(1).broadcast_to([128, 32, DIM]),
                op=ALU.mult)
            nc.vector.tensor_reduce(
                ret_sb[:, st * 32:(st + 1) * 32],
                TU[:].rearrange("p (m d) -> p m d", d=DIM),
                axis=AX.X, op=ALU.add)

        nc.sync.dma_start(ret2_sb[:], ret_sb[B:128, :])

        # ================= BN over z =================
        zT_ps = ps1.tile([128, 4 * B], BF16, tag="misc")
        for i in range(4):
            nc.tensor.transpose(zT_ps[:, i * B:(i + 1) * B],
                                z_sb[:, 128 * i:128 * (i + 1)], ident_sb[:])
        mu = sb.tile([128, 4], F32, tag="mu")
        msq = sb.tile([128, 4], F32, tag="msq")
        zsq = sb.tile([128, 4 * B], F32, tag="zsq")
        nc.scalar.square(zsq[:], zT_ps[:])
        for i in range(4):
            nc.vector.tensor_reduce(mu[:, i:i + 1], zT_ps[:, i * B:(i + 1) * B],
                                    axis=AX.X, op=ALU.add)
            nc.vector.tensor_reduce(msq[:, i:i + 1], zsq[:, i * B:(i + 1) * B],
                                    axis=AX.X, op=ALU.add)
        nc.vector.tensor_scalar(mu[:], mu[:], 1.0 / B, None, op0=ALU.mult)
        nc.vector.tensor_scalar(msq[:], msq[:], 1.0 / B, None, op0=ALU.mult)
        var = sb.tile([128, 4], F32, tag="var")
        nc.vector.tensor_tensor(var[:], mu[:], mu[:], op=ALU.mult)
        nc.vector.tensor_tensor(var[:], msq[:], var[:], op=ALU.subtract)
        nc.vector.tensor_scalar(var[:], var[:], BN_EPS, None, op0=ALU.add)
        std = sb.tile([128, 4], F32, tag="std")
        nc.scalar.sqrt(std[:], var[:])
        rstd = sb.tile([128, 4], F32, tag="rstd")
        nc.vector.reciprocal(rstd[:], std[:])
        scl = sb.tile([128, 4], F32, tag="scl")
        bia = sb.tile([128, 4], F32, tag="bia")
        gam_ap = gbT_sb[:].rearrange("p (c t) -> p c t", t=2)[:, :, 0]
        bet_ap = gbT_sb[:].rearrange("p (c t) -> p c t", t=2)[:, :, 1]
        nc.vector.tensor_tensor(scl[:], rstd[:], gam_ap, op=ALU.mult)
        nc.vector.tensor_tensor(bia[:], mu[:], scl[:], op=ALU.mult)
        nc.vector.tensor_tensor(bia[:], bet_ap, bia[:], op=ALU.subtract)
        for i in range(4):
            nc.scalar.activation(zbnT_sb[:, i * B:(i + 1) * B],
                                 zT_ps[:, i * B:(i + 1) * B],
                                 ACTF.Identity, bias=bia[:, i:i + 1],
                                 scale=scl[:, i:i + 1])

        # ================= decoder + ret + sigmoid =================
        for c in range(7):
            w = 512 if c < 6 else 128
            zd = ps.tile([B, 512], F32, tag="sd")
            for k in range(4):
                nc.tensor.matmul(zd[:, :w], zbnT_sb[:, k * B:(k + 1) * B],
                                 w0d_sb[:, k * VCP + c * 512: k * VCP + c * 512 + w],
                                 start=(k == 0), stop=False)
            nc.tensor.matmul(zd[:, :w], ones1_sb[:], db1_sb[:, c * 512:c * 512 + w],
                             start=False, stop=True)
            s0, s1 = 8 * c, min(8 * c + 8, NST)
            nsl = s1 - s0
            if nsl > 0:
                zv = zd[:, :w].rearrange("b (s x) -> b s x", x=64)
                nc.vector.tensor_tensor(
                    zv[:, :nsl, 0:32], zv[:, :nsl, 0:32],
                    ret_sb[0:B, s0 * 32:s1 * 32].rearrange("b (s j) -> b s j", j=32),
                    op=ALU.add)
                nc.vector.tensor_tensor(
                    zv[:, :nsl, 32:64], zv[:, :nsl, 32:64],
                    ret2_sb[:, s0 * 32:s1 * 32].rearrange("b (s j) -> b s j", j=32),
                    op=ALU.add)
            ob = sb.tile([B, 512], F32, tag="ob")
            nc.scalar.activation(ob[:, :w], zd[:, :w], ACTF.Sigmoid)
            nc.sync.dma_start(out_d[:, c * 512:c * 512 + w], ob[:, :w])

    nc.finalize()
    return nc


def _prep_inputs(inputs):
    x = np.asarray(inputs["x"], np.float32)
    w0 = np.asarray(inputs["enc_w0"], np.float32)
    b0 = np.asarray(inputs["enc_b0"], np.float32)
    w1 = np.asarray(inputs["enc_w1"], np.float32)
    b1 = np.asarray(inputs["enc_b1"], np.float32)
    db0 = np.asarray(inputs["dec_b0"], np.float32)
    db1 = np.asarray(inputs["dec_b1"], np.float32)
    gam = np.asarray(inputs["bn_gamma"], np.float32)
    bet = np.asarray(inputs["bn_beta"], np.float32)
    uw = np.asarray(inputs["u_w"], np.float32)
    ub = np.asarray(inputs["u_b"], np.float32)
    fcw = np.asarray(inputs["fc_w"], np.float32)
    fcb = np.asarray(inputs["fc_b"], np.float32)
    iemb = np.asarray(inputs["item_emb"], np.float32)
    ne = np.asarray(inputs["nbr_ent"], np.float32).reshape(NI, NN, DIM)
    nr = np.asarray(inputs["nbr_rel"], np.float32).reshape(NI, NN, DIM)

    fc1, fc2 = fcw[:, :DIM], fcw[:, DIM:]
    iproj = iemb @ fc1.T + fcb
    pp = ne @ fc2.T / NN + iproj[:, None, :]

    cols = []
    for c in range(NC):
        pc = list(range(NI + 35 * c, min(NV, NI + 35 * c + 35)))
        cols.append(list(range(MS * c, MS * (c + 1))) + pc)

    tobf = lambda a: np.ascontiguousarray(np.asarray(a, np.float32)).astype(ml_dtypes.bfloat16)
    gbT = np.stack([gam, bet], -1).reshape(4, 128, 2).astype(np.float32)
    sel = np.zeros((128, 128), np.float32)
    for m in range(32):
        sel[4 * m:4 * m + 4, 4 * m:4 * m + 4] = 1.0

    shared = {
        "w1T": tobf(w1.T), "b1r": tobf(b1.reshape(1, H2)),
        "w1": tobf(w1), "db0": tobf(db0.reshape(1, H1)),
        "uwT": tobf(uw.T), "ubr": tobf(ub.reshape(1, DIM)),
        "selc": tobf(sel), "ones1": tobf(np.ones((1, B), np.float32)),
        "ident": tobf(np.eye(B, dtype=np.float32)), "gbT": gbT,
    }

    in_maps = []
    for c in range(NC):
        cl = cols[c]
        ncd = len(cl)
        xs = np.zeros((VCP, B), np.float32)
        xs[:ncd] = x[:, cl].T
        xs[VCP - 1] = 1.0
        w0e = np.zeros((VCP, H1), np.float32)
        w0e[:ncd] = w0[:, cl].T
        w0e[VCP - 1] = b0 / NC
        w0dc = np.zeros((H1, VCP), np.float32)
        w0dc[:, :ncd] = w0[:, cl]
        db1c = np.zeros((1, VCP), np.float32)
        db1c[0, :ncd] = db1[cl]

        nrc = np.zeros((MSP, NN, DIM), np.float32)
        nrc[:MS] = nr[MS * c:MS * (c + 1)]
        ppc = np.zeros((MSP, NN, DIM), np.float32)
        ppc[:MS] = pp[MS * c:MS * (c + 1)]
        nrTc = nrc.reshape(MSP * NN, DIM).T
        # bdpd[q, h, j, n, t, d] = ppc[32*(GB*q+t) + 16*h + j, n, d]
        p6 = ppc.reshape(NB, GB, 2, 16, NN, DIM)
        bdpdc = np.ascontiguousarray(p6.transpose(0, 2, 3, 4, 1, 5))

        m = dict(shared)
        m["zrow"] = tobf(np.zeros((1, 1024), np.float32))
        m.update({
            "xT": tobf(xs), "w0e": tobf(w0e),
            "w0d": tobf(w0dc.reshape(4, 128, VCP)),
            "db1": tobf(db1c), "nrT": tobf(nrTc), "bdpd": tobf(bdpdc),
        })
        in_maps.append(m)
    return in_maps, cols


def kernel(**inputs) -> np.ndarray:
    if "nc" not in _CACHE:
        _CACHE["nc"] = _build_graph()
    nc = _CACHE["nc"]
    in_maps, cols = _prep_inputs(inputs)
    res = run_bass_kernel_spmd(nc, in_maps, core_ids=list(range(NC)))
    out = np.zeros((B, NV), np.float32)
    for c in range(NC):
        oc = res.results[c]["out"]
        cl = cols[c]
        out[:, cl] = oc[:, :len(cl)]
    return out


if __name__ == "__main__":
    sys.path.insert(0, "/root/problem")
    import reference
    ins = {k: np.asarray(v) for k, v in reference.setup_inputs().items()}
    exp = np.asarray(reference.reference(**ins))
    act = kernel(**ins)
    err = np.abs(act - exp).max() / (np.abs(exp).max() + 1e-9)
    print("Max abs err:", np.abs(act - exp).max(), " Relative error:", err)



# revision 2
# speedup vs baseline: 1.1934x; 1.1934x over previous
"""Trainium2 Bass kernel for nn_AE_KGCN (AE encoder + KGCN attention + tied decoder).

Sharding: items (25000) and enc_w0 vocab-columns (25274) are co-sharded over 8
cores (3125 items + ~3160 vocab cols each, padded to 3200). One AllReduce of the
[64,512] encoder partial sums is the only collective.

v2 layout notes:
  - w0 is shipped ONCE per core in encoder layout ([VCP,512] vocab-major);
    the decoder layout ([512,VCP]) is derived on-device with 4 XBAR
    transpose-DMAs during the AllReduce window.
  - Attention tables ship dense; the block-diagonal numer operand is built
    by SBUF->SBUF scatter from the resident dense copy (no tiny HBM
    descriptors). Zero backgrounds via memset compute ops, not DMA.
  - numer uses 16-item block-pairs: per 32-item tile one matmul
    [K=128]x[M=128]x[N=512]; lhsT is a 2x2-block En matrix (top items in
    cols 0:64, bottom items in cols 64:128), rhs is an 8-nonzero/column
    block-diag of P' = nbr_e@fc2.T/4 + iproj. PSUM rows 0:64 = items 0:16,
    rows 64:128 = items 16:32.
  - Output is downloaded bf16 and upcast on host.
"""

import sys

for p in ("/opt/trn_rl_repo", "/opt/pypackages"):
    if p not in sys.path:
        sys.path.insert(0, p)

import numpy as np
import ml_dtypes
import concourse.bass as bass
import concourse.mybir as mybir
import concourse.tile as tile
import concourse.bacc as bacc
from concourse.bass_utils import run_bass_kernel_spmd
from concourse.dve_ops import RECIPROCAL_APPROX_FAST, RECIP_APPROX_FAST_CONSTS

F32 = mybir.dt.float32
BF16 = mybir.dt.bfloat16
AX = mybir.AxisListType
ALU = mybir.AluOpType
ACTF = mybir.ActivationFunctionType

B = 64
NV = 25274
NI = 25000
DIM = 32
NN = 4
H1 = 512
H2 = 64
NC = 8
MS = NI // NC            # 3125 items per core
NT = 98                  # 32-item tiles per core
MSP = NT * 32            # 3136 padded items
VCP = 3200               # padded vocab cols per core (25*128)
NVCH = VCP // 128        # 25 encoder K-chunks
GB = 14                  # tiles per BD batch
NB = NT // GB            # 7 batches
SELU_L = 1.0507009873554805
SELU_A = 1.6732632423543772
BN_EPS = 1e-5

_CACHE = {}


def _build_graph():
    nc = bacc.Bacc("TRN2", target_bir_lowering=False, debug=False,
                   enable_asserts=False, num_devices=NC)

    def din(name, shape, dt=BF16):
        return nc.dram_tensor(name, shape, dt, kind="ExternalInput").ap()

    xT = din("xT", [128, NVCH, B])           # x shard, host pre-chunked transpose
    w0e = din("w0e", [VCP, H1])              # w0 shard .T (vocab-major)
    b0r = din("b0r", [1, H1])                # enc_b0 / NC row
    db1 = din("db1", [1, VCP])               # dec_b1 shard row
    w1Tc = din("w1Tc", [128, 4, H2])         # enc_w1.T, host pre-chunked
    b1r = din("b1r", [1, H2])
    w1 = din("w1", [H2, H1])
    db0 = din("db0", [1, H1])
    uwT = din("uwT", [H2, DIM])
    ubr = din("ubr", [1, DIM])
    nrT = din("nrT", [DIM, MSP * NN])        # nbr_rel shard [d, (m,n)]
    ppd = din("ppd", [128, NT * DIM])        # dense P' [(m%32,n), (tile,d)]
    selc = din("selc", [128, 128])           # block-diag 4x4 ones selector
    ones1 = din("ones1", [1, B])             # K=1 bias-row lhsT
    ident = din("ident", [B, B])             # identity for PE transpose
    gbTc = din("gbTc", [128, 4, 2], F32)     # [gamma, beta] per h1, chunked
    out_d = nc.dram_tensor("out", [B, VCP], BF16, kind="ExternalOutput").ap()

    from contextlib import ExitStack
    with tile.TileContext(nc) as tc, ExitStack() as ctx:
        sb = ctx.enter_context(tc.tile_pool(name="sb", bufs=2))
        sb3 = ctx.enter_context(tc.tile_pool(name="sb3", bufs=3))
        sb1 = ctx.enter_context(tc.tile_pool(name="sb1", bufs=1))
        ps = ctx.enter_context(tc.tile_pool(name="ps", bufs=3, space="PSUM"))
        ps1 = ctx.enter_context(tc.tile_pool(name="ps1", bufs=1, space="PSUM"))
        psm = ctx.enter_context(tc.tile_pool(name="psm", bufs=3, space="PSUM"))
        dram = ctx.enter_context(tc.tile_pool(name="dram", bufs=1, space="DRAM"))

        # ---- persistent SBUF ----
        xT_sb = sb1.tile([128, NVCH * B], BF16, tag="xT")
        w0d_sb = sb1.tile([128, 4 * VCP], BF16, tag="w0d")
        db1_sb = sb1.tile([1, VCP], BF16, tag="db1")
        w1T_sb = sb1.tile([128, 4 * H2], BF16, tag="w1T")
        w1_sb = sb1.tile([H2, H1], BF16, tag="w1")
        db0_sb = sb1.tile([1, H1], BF16, tag="db0")
        b0r_sb = sb1.tile([1, H1], BF16, tag="b0r")
        b1r_sb = sb1.tile([1, H2], BF16, tag="b1r")
        uwT_sb = sb1.tile([H2, DIM], BF16, tag="uwT")
        ubr_sb = sb1.tile([1, DIM], BF16, tag="ubr")
        nrT_sb = sb1.tile([DIM, MSP * NN], BF16, tag="nrT")
        ppd_sb = sb1.tile([128, NT * DIM], BF16, tag="ppd")
        selc_sb = sb1.tile([128, 128], BF16, tag="selc")
        ones1_sb = sb1.tile([1, B], BF16, tag="ones1")
        ident_sb = sb1.tile([B, B], BF16, tag="ident")
        gbT_sb = sb1.tile([128, 4 * 2], F32, tag="gbT")
        En2_sb = sb1.tile([128, NT * 128], BF16, tag="En2")
        bd0 = sb1.tile([128, GB * 512], BF16, tag="bd0")
        bd1 = sb1.tile([128, GB * 512], BF16, tag="bd1")
        bds = [bd0, bd1]
        ret_sb = sb1.tile([128, NT * 16], F32, tag="ret")
        ret2_sb = sb1.tile([B, NT * 16], F32, tag="ret2")
        hT_sb = sb1.tile([128, 4 * B], BF16, tag="hT")
        u2_sb = sb1.tile([128, DIM], BF16, tag="u2")
        usrT_sb = sb1.tile([DIM, B], BF16, tag="usrT")
        zbnT_sb = sb1.tile([128, 4 * B], BF16, tag="zbnT")

        # ---- zero backgrounds via compute (fills the launch-skew window) ----
        nc.vector.memset(En2_sb[:], 0.0)
        nc.gpsimd.memset(bd0[:], 0.0)
        nc.gpsimd.memset(bd1[:], 0.0)

        # ---- encoder-critical + tiny input DMAs first ----
        nc.sync.dma_start(xT_sb[:].rearrange("p (c b) -> p c b", b=B), xT)
        nc.sync.dma_start(w1T_sb[:].rearrange("p (c h) -> p c h", h=H2), w1Tc)
        for t, s in ((w1_sb, w1), (db0_sb, db0), (b0r_sb, b0r), (b1r_sb, b1r),
                     (uwT_sb, uwT), (ubr_sb, ubr), (selc_sb, selc),
                     (ones1_sb, ones1), (ident_sb, ident)):
            nc.sync.dma_start(t[:], s[:])
        nc.sync.dma_start(gbT_sb[:].rearrange("p (c t) -> p c t", t=2), gbTc)

        # ================= encoder =================
        h1ps = ps1.tile([B, H1], F32, tag="misc")
        w0ev = w0e.rearrange("(c p) h -> c p h", p=128)
        for v in range(NVCH):
            w0ec = sb3.tile([128, H1], BF16, tag="w0ec")
            nc.sync.dma_start(w0ec[:], w0ev[v])
            nc.tensor.matmul(
                h1ps[:], xT_sb[:, v * B:(v + 1) * B], w0ec[:],
                start=(v == 0), stop=False)
        nc.tensor.matmul(h1ps[:], ones1_sb[:], b0r_sb[:], start=False, stop=True)
        h1sb = sb.tile([B, H1], F32, tag="h1sb")
        nc.scalar.copy(h1sb[:], h1ps[:])

        bnc_in = dram.tile([B, H1], F32)
        bnc_out = dram.tile([B, H1], F32)
        nc.sync.dma_start(bnc_in[:], h1sb[:])
        nc.gpsimd.collective_compute(
            "AllReduce", ALU.add, replica_groups=[list(range(NC))],
            ins=[bnc_in.opt()], outs=[bnc_out.opt()])

        # ---- bulk loads: fire during the AllReduce window ----
        nc.sync.dma_start(nrT_sb[:], nrT[:])
        nc.sync.dma_start(ppd_sb[:], ppd[:])
        nc.sync.dma_start(db1_sb[:], db1[:])
        # decoder-layout w0 via XBAR transpose (reads w0e DRAM again)
        w0dv = w0d_sb[:].rearrange("p (k v) -> p k v", v=VCP)
        for k in range(4):
            nc.sync.dma_start(w0dv[:, k], w0e[:, k * 128:(k + 1) * 128],
                              transpose=True)

        def bd_scatter(q):
            buf = bds[q % 2]
            for h in range(2):
                for j in range(16):
                    p0 = 64 * h + 4 * j
                    dst = buf[p0:p0 + 4] \
                        .rearrange("n (i jj d) -> n i jj d", jj=16, d=DIM)[:, :, j, :]
                    src = ppd_sb[p0:p0 + 4] \
                        .rearrange("n (t d) -> n t d", d=DIM)[:, q * GB:(q + 1) * GB, :]
                    nc.sync.dma_start(dst, src)

        bd_scatter(0)
        bd_scatter(1)

        h1r = sb.tile([B, H1], F32, tag="h1r")
        nc.sync.dma_start(h1r[:], bnc_out[:])

        # selu helper: dst = SL*relu(x) + min(SA*SL*(exp(x)-1), 0)
        def selu(dst, src, P, W, tagp="sl"):
            e = sb.tile([P, W], F32, tag=tagp + "e")
            t = sb.tile([P, W], F32, tag=tagp + "t")
            f = sb.tile([P, W], F32, tag=tagp + "f")
            nc.scalar.activation(e[:], src, ACTF.Exp)
            nc.vector.tensor_scalar(t[:], src, SELU_L, 0.0, op0=ALU.mult, op1=ALU.max)
            nc.vector.tensor_scalar(f[:], e[:], SELU_A * SELU_L, -SELU_A * SELU_L,
                                    op0=ALU.mult, op1=ALU.add)
            nc.vector.tensor_scalar(f[:], f[:], 0.0, None, op0=ALU.min)
            nc.vector.tensor_tensor(dst, t[:], f[:], op=ALU.add)

        h_sb = sb.tile([B, H1], BF16, tag="h")
        selu(h_sb[:], h1r[:], B, H1)
        # hT via 4 PE transposes
        for i in range(4):
            htp = ps1.tile([128, B], BF16, tag="misc")
            nc.tensor.transpose(htp[:], h_sb[:, 128 * i:128 * (i + 1)], ident_sb[:])
            nc.scalar.copy(hT_sb[:, i * B:(i + 1) * B], htp[:])

        # ================= h2 / user / z =================
        h2ps = ps1.tile([B, H2], F32, tag="misc")
        for k in range(4):
            nc.tensor.matmul(h2ps[:], hT_sb[:, k * B:(k + 1) * B],
                             w1T_sb[:, k * H2:(k + 1) * H2],
                             start=(k == 0), stop=False)
        nc.tensor.matmul(h2ps[:], ones1_sb[:], b1r_sb[:], start=False, stop=True)
        h2s = sb.tile([B, H2], BF16, tag="h2s")
        selu(h2s[:], h2ps[:], B, H2)
        h2sT_ps = ps1.tile([H2, B], BF16, tag="misc")
        nc.tensor.transpose(h2sT_ps[:], h2s[:], ident_sb[:])
        h2sT = sb.tile([H2, B], BF16, tag="h2sTs")
        nc.scalar.copy(h2sT[:], h2sT_ps[:])

        usr_ps = ps1.tile([B, DIM], F32, tag="misc")
        nc.tensor.matmul(usr_ps[:], h2sT[:], uwT_sb[:], start=True, stop=False)
        nc.tensor.matmul(usr_ps[:], ones1_sb[:], ubr_sb[:], start=False, stop=True)
        usr_sb = sb.tile([B, DIM], BF16, tag="usrsb")
        nc.scalar.copy(usr_sb[:], usr_ps[:])
        usrT_ps = ps1.tile([DIM, B], BF16, tag="misc")
        nc.tensor.transpose(usrT_ps[:], usr_sb[:], ident_sb[:])
        nc.scalar.copy(usrT_sb[:], usrT_ps[:])
        nc.sync.dma_start(u2_sb[0:B, :], usr_sb[:])
        nc.sync.dma_start(u2_sb[B:128, :], usr_sb[:])

        zps = ps1.tile([B, H1], F32, tag="misc")
        nc.tensor.matmul(zps[:], h2sT[:], w1_sb[:], start=True, stop=False)
        nc.tensor.matmul(zps[:], ones1_sb[:], db0_sb[:], start=False, stop=True)
        z_sb = sb.tile([B, H1], BF16, tag="zsb")
        selu(z_sb[:], zps[:], B, H1)

        # ===== merged scores + softmax + numer pipeline (13 groups of 8 tiles) ==
        En2v = En2_sb[:].rearrange("p (t c) -> p t c", c=128)
        next_scatter = 2
        for g in range(13):
            t0, t1 = g * 8, min(g * 8 + 8, NT)
            ntl = t1 - t0
            sps = ps.tile([128, 512], F32, tag="sd")
            for t in range(t0, t1):
                nc.tensor.matmul(sps[:, (t - t0) * B:(t - t0 + 1) * B],
                                 nrT_sb[:, t * 128:(t + 1) * 128], usrT_sb[:],
                                 start=True, stop=True)
            Eg = sb3.tile([128, 512], BF16, tag="Eg")
            nc.scalar.activation(Eg[:, :ntl * B], sps[:, :ntl * B],
                                 ACTF.Exp, scale=1.0 / DIM)
            dps = ps.tile([128, 512], F32, tag="sd")
            for t in range(t0, t1):
                nc.tensor.matmul(dps[:, (t - t0) * B:(t - t0 + 1) * B],
                                 selc_sb[:], Eg[:, (t - t0) * B:(t - t0 + 1) * B],
                                 start=True, stop=True)
            rcpg = sb3.tile([128, 512], BF16, tag="rcpg")
            nc.vector._custom_dve(
                RECIPROCAL_APPROX_FAST,
                out=rcpg[:, :ntl * B], in0=dps[:, :ntl * B],
                s0=RECIP_APPROX_FAST_CONSTS["s0"], s1=RECIP_APPROX_FAST_CONSTS["s1"],
                imm2=RECIP_APPROX_FAST_CONSTS["imm2"])
            Egv = Eg[:, :ntl * B].rearrange("p (t b) -> p t b", b=B)
            rcv = rcpg[:, :ntl * B].rearrange("p (t b) -> p t b", b=B)
            nc.vector.tensor_tensor(En2v[0:B, t0:t1, 0:B],
                                    Egv[0:B], rcv[0:B], op=ALU.mult)
            nc.gpsimd.tensor_tensor(En2v[B:128, t0:t1, B:128],
                                    Egv[B:128], rcv[B:128], op=ALU.mult)
            # numer + tanh + TU + reduce for this group's tiles
            for t in range(t0, t1):
                q, i = divmod(t, GB)
                if i == 0 and q + 1 == next_scatter and next_scatter < NB:
                    bd_scatter(next_scatter)
                    next_scatter += 1
                nps = psm.tile([128, 512], F32, tag="nps")
                nc.tensor.matmul(nps[:], En2_sb[:, t * 128:(t + 1) * 128],
                                 bds[q % 2][:, i * 512:(i + 1) * 512],
                                 start=True, stop=True)
                T_sb = sb3.tile([128, 512], BF16, tag="T")
                nc.scalar.activation(T_sb[:], nps[:], ACTF.Tanh)
                TU = sb3.tile([128, 512], BF16, tag="TU")
                tt_eng = nc.vector if (t % 2 == 0) else nc.gpsimd
                tt_eng.tensor_tensor(
                    TU[:].rearrange("p (m d) -> p m d", d=DIM),
                    T_sb[:].rearrange("p (m d) -> p m d", d=DIM),
                    u2_sb[:].unsqueeze(1).broadcast_to([128, 16, DIM]),
                    op=ALU.mult)
                nc.vector.tensor_reduce(
                    ret_sb[:, t * 16:(t + 1) * 16],
                    TU[:].rearrange("p (m d) -> p m d", d=DIM),
                    axis=AX.X, op=ALU.add)

        nc.sync.dma_start(ret2_sb[:], ret_sb[B:128, :])

        # ================= BN over z =================
        zT_ps = ps1.tile([128, 4 * B], BF16, tag="misc")
        for i in range(4):
            nc.tensor.transpose(zT_ps[:, i * B:(i + 1) * B],
                                z_sb[:, 128 * i:128 * (i + 1)], ident_sb[:])
        mu = sb.tile([128, 4], F32, tag="mu")
        msq = sb.tile([128, 4], F32, tag="msq")
        zsq = sb.tile([128, 4 * B], F32, tag="zsq")
        nc.scalar.square(zsq[:], zT_ps[:])
        for i in range(4):
            nc.vector.tensor_reduce(mu[:, i:i + 1], zT_ps[:, i * B:(i + 1) * B],
                                    axis=AX.X, op=ALU.add)
            nc.vector.tensor_reduce(msq[:, i:i + 1], zsq[:, i * B:(i + 1) * B],
                                    axis=AX.X, op=ALU.add)
        nc.vector.tensor_scalar(mu[:], mu[:], 1.0 / B, None, op0=ALU.mult)
        nc.vector.tensor_scalar(msq[:], msq[:], 1.0 / B, None, op0=ALU.mult)
        var = sb.tile([128, 4], F32, tag="var")
        nc.vector.tensor_tensor(var[:], mu[:], mu[:], op=ALU.mult)
        nc.vector.tensor_tensor(var[:], msq[:], var[:], op=ALU.subtract)
        nc.vector.tensor_scalar(var[:], var[:], BN_EPS, None, op0=ALU.add)
        std = sb.tile([128, 4], F32, tag="std")
        nc.scalar.sqrt(std[:], var[:])
        rstd = sb.tile([128, 4], F32, tag="rstd")
        nc.vector.reciprocal(rstd[:], std[:])
        scl = sb.tile([128, 4], F32, tag="scl")
        bia = sb.tile([128, 4], F32, tag="bia")
        gam_ap = gbT_sb[:].rearrange("p (c t) -> p c t", t=2)[:, :, 0]
        bet_ap = gbT_sb[:].rearrange("p (c t) -> p c t", t=2)[:, :, 1]
        nc.vector.tensor_tensor(scl[:], rstd[:], gam_ap, op=ALU.mult)
        nc.vector.tensor_tensor(bia[:], mu[:], scl[:], op=ALU.mult)
        nc.vector.tensor_tensor(bia[:], bet_ap, bia[:], op=ALU.subtract)
        for i in range(4):
            nc.scalar.activation(zbnT_sb[:, i * B:(i + 1) * B],
                                 zT_ps[:, i * B:(i + 1) * B],
                                 ACTF.Identity, bias=bia[:, i:i + 1],
                                 scale=scl[:, i:i + 1])

        # ================= decoder + ret + sigmoid =================
        retv = ret_sb[0:B, :].rearrange("b (t j) -> b t j", j=16)
        ret2v = ret2_sb[:].rearrange("b (t j) -> b t j", j=16)
        for c in range(7):
            w = 512 if c < 6 else 128
            zd = ps.tile([B, 512], F32, tag="sd")
            for k in range(4):
                nc.tensor.matmul(zd[:, :w], zbnT_sb[:, k * B:(k + 1) * B],
                                 w0d_sb[:, k * VCP + c * 512: k * VCP + c * 512 + w],
                                 start=(k == 0), stop=False)
            nc.tensor.matmul(zd[:, :w], ones1_sb[:], db1_sb[:, c * 512:c * 512 + w],
                             start=False, stop=True)
            tt0 = 16 * c
            ntt = min(16, NT - 16 * c)
            zv = zd[:, :w].rearrange("b (t h j) -> b t h j", h=2, j=16)
            nc.vector.tensor_tensor(zv[:, :ntt, 0, :], zv[:, :ntt, 0, :],
                                    retv[:, tt0:tt0 + ntt, :], op=ALU.add)
            nc.vector.tensor_tensor(zv[:, :ntt, 1, :], zv[:, :ntt, 1, :],
                                    ret2v[:, tt0:tt0 + ntt, :], op=ALU.add)
            ob = sb.tile([B, 512], BF16, tag="ob")
            nc.scalar.activation(ob[:, :w], zd[:, :w], ACTF.Sigmoid)
            nc.sync.dma_start(out_d[:, c * 512:c * 512 + w], ob[:, :w])

    nc.finalize()
    return nc


def _shard_cols(c):
    p0 = NI + 35 * c
    p1 = min(NV, p0 + 35)
    return p0, p1


def _prep_inputs(inputs):
    bf = ml_dtypes.bfloat16
    x = np.asarray(inputs["x"], np.float32)
    w0 = np.asarray(inputs["enc_w0"], np.float32)
    b0 = np.asarray(inputs["enc_b0"], np.float32)
    w1 = np.asarray(inputs["enc_w1"], np.float32)
    b1 = np.asarray(inputs["enc_b1"], np.float32)
    db0 = np.asarray(inputs["dec_b0"], np.float32)
    db1 = np.asarray(inputs["dec_b1"], np.float32)
    gam = np.asarray(inputs["bn_gamma"], np.float32)
    bet = np.asarray(inputs["bn_beta"], np.float32)
    uw = np.asarray(inputs["u_w"], np.float32)
    ub = np.asarray(inputs["u_b"], np.float32)
    fcw = np.asarray(inputs["fc_w"], np.float32)
    fcb = np.asarray(inputs["fc_b"], np.float32)
    iemb = np.asarray(inputs["item_emb"], np.float32)
    ne = np.asarray(inputs["nbr_ent"], np.float32)
    nr = np.asarray(inputs["nbr_rel"], np.float32)

    fc1, fc2 = fcw[:, :DIM], fcw[:, DIM:]
    iproj = iemb @ fc1.T + fcb
    pp = (ne @ (fc2.T / NN)).reshape(NI, NN, DIM) + iproj[:, None, :]
    nr = nr.reshape(NI, NN, DIM)

    # one-shot bf16 conversions of the big tables, sliced per core afterwards
    w0b = w0.astype(bf)                      # [H1, NV]
    xb = x.astype(bf)                        # [B, NV]
    ppb = pp.astype(bf)
    nrb = nr.astype(bf)

    gbTc = np.ascontiguousarray(
        np.stack([gam, bet], -1).reshape(4, 128, 2).transpose(1, 0, 2)
    ).astype(np.float32)
    sel = np.zeros((128, 128), np.float32)
    for m in range(32):
        sel[4 * m:4 * m + 4, 4 * m:4 * m + 4] = 1.0

    tobf = lambda a: np.ascontiguousarray(np.asarray(a, np.float32)).astype(bf)
    shared = {
        "w1Tc": np.ascontiguousarray(
            w1.T.reshape(4, 128, H2).transpose(1, 0, 2).astype(bf)),
        "b1r": tobf(b1.reshape(1, H2)),
        "w1": tobf(w1), "db0": tobf(db0.reshape(1, H1)),
        "b0r": tobf((b0 / NC).reshape(1, H1)),
        "uwT": tobf(uw.T), "ubr": tobf(ub.reshape(1, DIM)),
        "selc": tobf(sel), "ones1": tobf(np.ones((1, B), np.float32)),
        "ident": tobf(np.eye(B, dtype=np.float32)), "gbTc": gbTc,
    }

    in_maps = []
    col_ranges = []
    for c in range(NC):
        p0, p1 = _shard_cols(c)
        npc = p1 - p0
        ncd = MS + npc
        col_ranges.append((MS * c, MS * (c + 1), p0, p1))

        # xT chunks [128, NVCH, B]
        xs = np.zeros((VCP, B), bf)
        xs[:MS] = xb[:, MS * c:MS * (c + 1)].T
        xs[MS:ncd] = xb[:, p0:p1].T
        xTc = np.ascontiguousarray(xs.reshape(NVCH, 128, B).transpose(1, 0, 2))

        # w0 shard, encoder layout [VCP, H1]
        w0ec = np.zeros((VCP, H1), bf)
        w0ec[:MS] = w0b[:, MS * c:MS * (c + 1)].T
        w0ec[MS:ncd] = w0b[:, p0:p1].T

        db1c = np.zeros((1, VCP), bf)
        db1c[0, :MS] = db1[MS * c:MS * (c + 1)].astype(bf)
        db1c[0, MS:ncd] = db1[p0:p1].astype(bf)

        nrc = np.zeros((MSP, NN, DIM), bf)
        nrc[:MS] = nrb[MS * c:MS * (c + 1)]
        nrTc = np.ascontiguousarray(nrc.reshape(MSP * NN, DIM).T)

        ppc = np.zeros((MSP, NN, DIM), bf)
        ppc[:MS] = ppb[MS * c:MS * (c + 1)]
        # dense P': partition (m%32)*4+n, col (tile, d)
        ppdc = np.ascontiguousarray(
            ppc.reshape(NT, 32, NN, DIM).transpose(1, 2, 0, 3).reshape(128, NT * DIM))

        m = dict(shared)
        m.update({
            "xT": xTc, "w0e": w0ec, "db1": db1c,
            "nrT": nrTc, "ppd": ppdc,
        })
        in_maps.append(m)
    return in_maps, col_ranges


def kernel(**inputs) -> np.ndarray:
    if "nc" not in _CACHE:
        _CACHE["nc"] = _build_graph()
    nc = _CACHE["nc"]
    in_maps, col_ranges = _prep_inputs(inputs)
    res = run_bass_kernel_spmd(nc, in_maps, core_ids=list(range(NC)))
    out = np.zeros((B, NV), np.float32)
    for c in range(NC):
        oc = np.asarray(res.results[c]["out"]).astype(np.float32)
        m0, m1, p0, p1 = col_ranges[c]
        out[:, m0:m1] = oc[:, :MS]
        out[:, p0:p1] = oc[:, MS:MS + (p1 - p0)]
    return out


if __name__ == "__main__":
    sys.path.insert(0, "/root/problem")
    import reference
    ins = {k: np.asarray(v) for k, v in reference.setup_inputs().items()}
    exp = np.asarray(reference.reference(**ins))
    act = kernel(**ins)
    err = np.abs(act - exp).max() / (np.abs(exp).max() + 1e-9)
    print("Max abs err:", np.abs(act - exp).max(), " Relative error:", err)
